# revision 4
# baseline (speedup 1.0000x reference)
"""MaxPool3d kernel v28: bf16, parity-split DMA rings, chunked edge tiles.

Per-core work: 8 channels, 32 MiB bf16 loads + 4 MiB stores through 16
SDMA engines at ~26.9 GB/s each, HBM-paced. Clean-core budget: preamble
~2.1us, descriptor span ~105us, runtime epilogue ~6.3us.

- Host casts f32->bf16 (rounding rel-err <= 2^-8, far under the 2e-2
  gate); device streams half the bytes; output upcast to f32 on host.
- Even-parity plane loads ride the SP HWDGE ring, odd-parity the ACT
  ring: the two read streams interleave within the same channel (better
  HBM locality than alternating whole channels per ring, measured ~2us
  per core) and a tile's parity pair lands in parallel.
- Tiles 0 and NT-1 split into 4 row-quarters (loads, DVE chain, and for
  the last tile also stores, alternating rings) so DVE starts ~5us in
  and the post-last-load drain is short; mid tiles in halves so DVE is
  free when the last tile's quarters arrive.
- Each in-flight chunk DMA has its OWN semaphore: per-engine increments
  of concurrent DMAs interleave, so cumulative counts on a shared sem
  would fire before a slow engine finished the watched chunk.
- DVE order per tile: d-max (contiguous, 2x mode), h-max (2x), w-max
  (1x); ~9us/tile, under the ~11us/tile DMA pace.
- wm ring of 4 buffers: w(t) waits store(t-4), which has long drained,
  so DVE is never chained to store completion.

Partition p = 2*d''+hh holds rows [64hh, 64hh+64) of planes {2d''+par};
per-partition DRAM chunks are contiguous 16 KiB (8 KiB for halved
tiles), the efficient descriptor size. Stores ride alternating rings two
tiles behind. Known residual: a roaming ~18%-slower SDMA engine appears
on 0-4 cores per run and adds ~15us to an affected core; per-byte
slowdown, not fixable by descriptor shaping.
"""

import ml_dtypes
import numpy as np

import concourse.bass as bass
from concourse import bacc, mybir
from concourse import bass_utils

CPC = 8
D = H = W = 128
DT = mybir.dt.bfloat16
NSLOT = 4
NT = 8
NCHUNK = {0: 4, NT - 1: 4}  # quarters for first/last tile, halves otherwise
NCHUNK_DEFAULT = 2

_CACHE = {}


def _build_module():
    nc = bacc.Bacc("TRN2", target_bir_lowering=False, debug=False, num_devices=8)
    x = nc.dram_tensor("x", [CPC, D, H, W], DT, kind="ExternalInput").ap()
    y = nc.dram_tensor("y", [CPC, D // 2, H // 2, W // 2], DT, kind="ExternalOutput").ap()

    a0 = [nc.alloc_sbuf_tensor(f"a0_{i}", [128, 64, 128], DT).ap() for i in range(NSLOT)]
    a1 = [nc.alloc_sbuf_tensor(f"a1_{i}", [128, 64, 128], DT).ap() for i in range(NSLOT)]
    ab = nc.alloc_sbuf_tensor("ab", [128, 64, 128], DT).ap()
    hm = nc.alloc_sbuf_tensor("hm", [128, 32, 128], DT).ap()
    wm = [nc.alloc_sbuf_tensor(f"wm_{i}", [128, 32, 64], DT).ap() for i in range(4)]

    # chunked tiles have several in-flight DMAs: each chunk needs its OWN
    # semaphore (per-engine increments of concurrent DMAs interleave, so a
    # cumulative count on one sem can hit the threshold while a slow
    # engine's descriptor for the watched chunk is still outstanding).
    # Mid tiles: per-(slot, half) sems; first/last tile: per-quarter sems
    # (tiles 0 and NT-1 are never in flight together).
    a0h_sems = [
        [nc.alloc_semaphore(f"a0h_sem{i}_{j}") for j in range(2)]
        for i in range(NSLOT)
    ]
    a1h_sems = [
        [nc.alloc_semaphore(f"a1h_sem{i}_{j}") for j in range(2)]
        for i in range(NSLOT)
    ]
    qa0_sems = [nc.alloc_semaphore(f"qa0_sem{i}") for i in range(4)]
    qa1_sems = [nc.alloc_semaphore(f"qa1_sem{i}") for i in range(4)]
    wm_sems = [nc.alloc_semaphore(f"wm_sem{i}") for i in range(4)]
    rel_sem = nc.alloc_semaphore("rel_sem")
    comp_sem = nc.alloc_semaphore("comp_sem")

    nchunk = [NCHUNK.get(t, NCHUNK_DEFAULT) for t in range(NT)]
    cumw = [0] * (NT + 1)
    for t in range(NT):
        cumw[t + 1] = cumw[t] + nchunk[t]

    hsem_base = [[0, 0] for _ in range(NSLOT)]
    qsem_base = [0] * 4
    load_thresh = {}  # t -> per-chunk (sem0, sem1, target)

    def ldeng(t):
        return nc.sync if t % 2 == 0 else nc.scalar

    def steng(t):
        return nc.scalar if t % 2 == 0 else nc.sync

    def emit_load(t):
        k = t % NSLOT
        # even-parity planes ride the SP ring, odd-parity the ACT ring:
        # the two streams read interleaved regions of the same channel
        # (better HBM locality) and a tile's parity pair lands in parallel
        Be = x[t, 0:D:2].rearrange("d (hh r) w -> d hh r w", hh=2)
        Bo = x[t, 1:D:2].rearrange("d (hh r) w -> d hh r w", hh=2)
        if t >= NSLOT:
            nc.sync.wait_ge(rel_sem, t - NSLOT + 1)
            nc.scalar.wait_ge(rel_sem, t - NSLOT + 1)
        nq = nchunk[t]
        rows = 64 // nq
        load_thresh[t] = []
        for q in range(nq):
            r0, r1 = rows * q, rows * (q + 1)
            if nq == 4:
                s0, s1 = qa0_sems[q], qa1_sems[q]
                thresh = qsem_base[q] + 16
                qsem_base[q] += 16
            else:
                s0, s1 = a0h_sems[k][q], a1h_sems[k][q]
                thresh = hsem_base[k][q] + 16
                hsem_base[k][q] += 16
            nc.sync.dma_start(a0[k][:, r0:r1, :], Be[:, :, r0:r1, :]).then_inc(
                s0, 16
            )
            nc.scalar.dma_start(a1[k][:, r0:r1, :], Bo[:, :, r0:r1, :]).then_inc(
                s1, 16
            )
            load_thresh[t].append((s0, s1, thresh))

    def emit_store(t):
        m = t % 4
        eng = steng(t)
        if t < NT - 1:
            eng.wait_ge(comp_sem, cumw[t + 1])
            eng.dma_start(y[t], wm[m]).then_inc(wm_sems[m], 16)
        else:
            oth = nc.scalar if eng is nc.sync else nc.sync
            yv = y[t].rearrange("d (hh r) w -> d hh r w", hh=2)
            for q in range(nchunk[t]):
                r0, r1 = 8 * q, 8 * q + 8
                seng = eng if q % 2 == 0 else oth
                seng.wait_ge(comp_sem, cumw[t] + q + 1)
                seng.dma_start(
                    yv[:, :, r0:r1, :], wm[m][:, r0:r1, :]
                ).then_inc(wm_sems[m], 16)

    for t in range(NT):
        emit_load(t)
        if t >= 2:
            emit_store(t - 2)
    emit_store(NT - 2)
    emit_store(NT - 1)

    # --- DVE: d-max (contiguous 2x) + h-max (2x) + w-max (1x) ----------
    wp = hm.rearrange("p r (w2 two) -> p r w2 two", two=2)
    for t in range(NT):
        k = t % NSLOT
        m = t % 4
        if t >= 4:
            nc.vector.wait_ge(wm_sems[m], 16 * (t // 4))
        nq = nchunk[t]
        rows = 64 // nq
        for q in range(nq):
            r0, r1 = rows * q, rows * (q + 1)
            h0, h1 = r0 // 2, r1 // 2
            s0, s1, thresh = load_thresh[t][q]
            nc.vector.wait_ge(s0, thresh)
            nc.vector.wait_ge(s1, thresh)
            dq = nc.vector.tensor_max(
                ab[:, r0:r1, :], a0[k][:, r0:r1, :], a1[k][:, r0:r1, :]
            )
            if q == nq - 1:
                dq.then_inc(rel_sem, 1)
            nc.vector.tensor_max(
                hm[:, h0:h1, :], ab[:, r0:r1:2, :], ab[:, r0 + 1 : r1 : 2, :]
            )
            nc.vector.tensor_max(
                wm[m][:, h0:h1, :], wp[:, h0:h1, :, 0], wp[:, h0:h1, :, 1]
            ).then_inc(comp_sem, 1)

    nc.compile()
    return nc


def _get_module():
    if "nc" not in _CACHE:
        _CACHE["nc"] = _build_module()
    return _CACHE["nc"]


def _prep(xf: np.ndarray) -> list:
    xb = xf.astype(ml_dtypes.bfloat16)
    return [
        {"x": np.ascontiguousarray(xb[i * CPC : (i + 1) * CPC])} for i in range(8)
    ]


def _post(ys: list) -> np.ndarray:
    return np.concatenate(ys, axis=0).astype(np.float32)


def kernel(x: np.ndarray) -> np.ndarray:
    B, C, d, h, w = x.shape
    assert (B, C, d, h, w) == (2, 32, 128, 128, 128), x.shape
    nc = _get_module()

    xf = np.asarray(x, dtype=np.float32).reshape(B * C, d, h, w)
    in_maps = _prep(xf)
    res = bass_utils.run_bass_kernel_spmd(nc, in_maps, core_ids=list(range(8)))
    return _post([r["y"] for r in res.results]).reshape(B, C, d // 2, h // 2, w // 2)
